# revision 22
# baseline (speedup 1.0000x reference)
"""Multi-head attention (embed 1024, 16 heads x 64) on 8 TRN2 NeuronCores.

Sharding: tensor-parallel over heads — each core owns 2 heads end-to-end
(qkv projection columns + attention), then per-batch AllToAlls redistribute
the per-head attention outputs so each core computes the out-projection for
its 256-token slice of each batch.

Compute is bf16 on the TensorEngine (fp32 PSUM accumulation), which keeps
the PE MAC-dense (HAM stays at full clock), makes weight loads FWL-eligible,
and halves DMA/collective bytes. Layout choices:
  - x is PE-transposed to xT [e, t] so every projection contracts e on
    partitions; projections produce Q/K/V TRANSPOSED [head_dim, t].
  - scores are computed transposed: St[tk, tq] = Kt.T @ Qt (one bf16 matmul
    per tk-tile, N=1024, single-bank bf16 PSUM), so the softmax sum over tk
    comes free from a ones-column appended to V: the PV matmul outputs
    [65, tq] with row 64 = sum of exp.
  - normalization: reciprocal of row 64, broadcast across 64 partitions with
    a K=1 matmul, multiplied in on the vector engine.
  - out_proj consumes the AllToAll output directly (head-dim on partitions)
    and the result is PE-transposed back to row-major before the output DMA.
"""

import numpy as np
import ml_dtypes

import concourse.bass as bass
import concourse.tile as tile
from concourse import bacc, mybir
from concourse.bass_utils import run_bass_kernel_spmd
from concourse.masks import make_identity

N_CORES = 8
B, S, D = 2, 2048, 1024
T = B * S              # 4096 flattened tokens
HEADS = 16
DH = 64                # head dim
HPC = HEADS // N_CORES  # heads per core = 2
CW = HPC * DH          # per-core qkv width = 128
SCALE = DH ** -0.5
TC = T // N_CORES      # per-core output rows = 512 (256 per batch)
NW = TC // 2           # tokens per batch slice = 256
ET = D // 128          # e partition tiles = 8
F32 = mybir.dt.float32
BF16 = mybir.dt.bfloat16
EXP = mybir.ActivationFunctionType.Exp
BF = ml_dtypes.bfloat16

_CACHED_NC = None


def build():
    nc = bacc.Bacc(
        "TRN2",
        target_bir_lowering=False,
        debug=False,
        num_devices=N_CORES,
    )
    x_ap = nc.dram_tensor("x", [T, D], BF16, kind="ExternalInput").ap()
    wq_ap = nc.dram_tensor("wq", [D, CW], BF16, kind="ExternalInput").ap()
    wk_ap = nc.dram_tensor("wk", [D, CW], BF16, kind="ExternalInput").ap()
    wv_ap = nc.dram_tensor("wv", [D, CW], BF16, kind="ExternalInput").ap()
    bq_ap = nc.dram_tensor("bq", [CW, 1], F32, kind="ExternalInput").ap()
    bk_ap = nc.dram_tensor("bk", [CW, 1], F32, kind="ExternalInput").ap()
    bv_ap = nc.dram_tensor("bv", [CW, 1], F32, kind="ExternalInput").ap()
    wout_ap = nc.dram_tensor("wout", [D, D], BF16, kind="ExternalInput").ap()
    bout_ap = nc.dram_tensor("bout", [128, ET], F32, kind="ExternalInput").ap()
    out_ap = nc.dram_tensor("out", [TC, D], F32, kind="ExternalOutput").ap()

    with tile.TileContext(nc) as tc:
        with (
            tc.tile_pool(name="singles", bufs=1) as singles,
            tc.tile_pool(name="xn", bufs=3) as xn_pool,
            tc.tile_pool(name="xt", bufs=2) as xt_pool,
            tc.tile_pool(name="vt", bufs=2) as vt_pool,
            tc.tile_pool(name="exp", bufs=4) as exp_pool,
            tc.tile_pool(name="fo", bufs=2) as fo_pool,
            tc.tile_pool(name="wo", bufs=2) as wo_pool,
            tc.tile_pool(name="small", bufs=2) as small_pool,
            tc.tile_pool(name="mmps", bufs=2, space="PSUM") as mmps,
            tc.tile_pool(name="stps", bufs=1, space="PSUM") as stps,
            tc.tile_pool(name="pvps", bufs=2, space="PSUM") as pvps,
            tc.tile_pool(name="dram", bufs=1, space="DRAM") as dram,
        ):
            # ---- A2A bounce buffers, one pair per batch (collectives need
            # internal DRAM). Shard j = tokens [j*256,(j+1)*256) of batch b.
            a2a_in = [dram.tile([D, NW], BF16, name=f"a2a_in{b}") for b in range(2)]
            a2a_out = [dram.tile([D, NW], BF16, name=f"a2a_out{b}") for b in range(2)]

            # ---- constants / weights resident in SBUF ----
            identb = singles.tile([128, 128], BF16)
            make_identity(nc, identb)
            ident32 = singles.tile([128, 128], F32)
            make_identity(nc, ident32)
            ones64 = singles.tile([1, DH], BF16)
            nc.vector.memset(ones64, 1.0)

            w_sb, b_sb = {}, {}
            for name, wap, bap in (
                ("q", wq_ap, bq_ap), ("k", wk_ap, bk_ap), ("v", wv_ap, bv_ap)
            ):
                w_sb[name] = singles.tile(
                    [128, ET, CW], BF16, tag=f"w{name}", name=f"w{name}_sb"
                )
                nc.sync.dma_start(
                    out=w_sb[name],
                    in_=wap.rearrange("(et p) c -> p et c", p=128),
                )
                b_sb[name] = singles.tile(
                    [CW, 1], F32, tag=f"b{name}", name=f"b{name}_sb"
                )
                nc.sync.dma_start(out=b_sb[name], in_=bap)
            bout_sb = singles.tile([128, ET], F32)
            nc.sync.dma_start(out=bout_sb, in_=bout_ap)

            # persistent activations
            qt = singles.tile([CW, T], BF16, tag="qt")   # [2h*64, t] transposed Q
            kt = singles.tile([CW, T], BF16, tag="kt")
            # V natural per head, 65-wide tk-tiles (col 64 = ones for denom)
            vsb = [
                singles.tile(
                    [128, T // 128, DH + 1], BF16, tag=f"v{h}", name=f"v{h}_sb"
                )
                for h in range(HPC)
            ]
            for h in range(HPC):
                nc.vector.memset(vsb[h][:, :, DH:DH + 1], 1.0)
            # gathered head-features for this core's token rows, per batch
            g_sb = [
                singles.tile([128, ET, NW], BF16, tag=f"g{b}", name=f"g{b}_sb")
                for b in range(2)
            ]

            # ---- stage A: transpose x + qkv projections (per 512-token chunk)
            for tch in range(T // 512):
                xt_sb = xt_pool.tile([128, ET, 512], BF16)
                for tt in range(4):
                    xn = xn_pool.tile([128, D], BF16)
                    row0 = tch * 512 + tt * 128
                    nc.sync.dma_start(out=xn, in_=x_ap[row0:row0 + 128, :])
                    for et in range(ET):
                        ps = mmps.tile([128, 128], BF16, tag="mm", name="ps_tr")
                        nc.tensor.transpose(
                            ps, xn[:, et * 128:(et + 1) * 128], identb
                        )
                        nc.vector.tensor_copy(
                            xt_sb[:, et, tt * 128:(tt + 1) * 128], ps
                        )
                for name, dest in (("q", qt), ("k", kt), ("v", None)):
                    pp = mmps.tile([CW, 512], F32, tag="mm", name="pp_proj")
                    for et in range(ET):
                        nc.tensor.matmul(
                            pp,
                            w_sb[name][:, et, :],
                            xt_sb[:, et, :],
                            start=(et == 0),
                            stop=(et == ET - 1),
                        )
                    if dest is not None:
                        nc.vector.tensor_scalar_add(
                            dest[:, tch * 512:(tch + 1) * 512], pp, b_sb[name]
                        )
                    else:
                        vt_tmp = vt_pool.tile([CW, 512], BF16)
                        nc.vector.tensor_scalar_add(vt_tmp, pp, b_sb[name])
                        for tt in range(4):
                            ps2 = mmps.tile([128, 128], BF16, tag="mm", name="ps_vtr")
                            nc.tensor.transpose(
                                ps2, vt_tmp[:, tt * 128:(tt + 1) * 128], identb
                            )
                            ttg = tch * 4 + tt
                            for h in range(HPC):
                                nc.vector.tensor_copy(
                                    vsb[h][:, ttg, 0:DH],
                                    ps2[:, h * DH:(h + 1) * DH],
                                )

                # ---- stage B: attention for batch b once its chunks are done
                if tch == 3 or tch == 7:
                    b = tch // 4
                    for h in range(HPC):
                        po = h * DH
                        for tqh in range(2):  # 1024-wide tq slabs
                            tq0 = b * S + tqh * 1024
                            pv = pvps.tile([DH + 1, 1024], F32)
                            for tkt in range(16):
                                st = stps.tile([128, 1024], F32, tag="st")
                                k0 = b * S + tkt * 128
                                for nh in range(2):
                                    nc.tensor.matmul(
                                        st[:, nh * 512:(nh + 1) * 512],
                                        kt[po:po + DH, k0:k0 + 128],
                                        qt[po:po + DH,
                                           tq0 + nh * 512:tq0 + (nh + 1) * 512],
                                    )
                                ex = exp_pool.tile([128, 1024], BF16)
                                nc.scalar.activation(ex, st, EXP)
                                for nh in range(2):
                                    nc.tensor.matmul(
                                        pv[:, nh * 512:(nh + 1) * 512],
                                        vsb[h][:, b * 16 + tkt, :],
                                        ex[:, nh * 512:(nh + 1) * 512],
                                        start=(tkt == 0),
                                        stop=(tkt == 15),
                                    )
                            recip = small_pool.tile([1, 1024], BF16, name="recip")
                            with nc.allow_low_precision(
                                reason="softmax denom reciprocal feeds bf16 bcast"
                            ):
                                nc.vector.reciprocal(recip, pv[DH:DH + 1, :])
                            pvc = fo_pool.tile([DH, 1024], F32, tag="pvc", name="pvc")
                            nc.vector.tensor_copy(pvc, pv[0:DH, :])
                            fo = fo_pool.tile([DH, 1024], BF16)
                            for nh in range(2):
                                bc = mmps.tile([DH, 512], F32, tag="mm", name="bc")
                                nc.tensor.matmul(
                                    bc,
                                    ones64,
                                    recip[:, nh * 512:(nh + 1) * 512],
                                )
                                nc.vector.tensor_mul(
                                    fo[:, nh * 512:(nh + 1) * 512],
                                    pvc[:, nh * 512:(nh + 1) * 512],
                                    bc,
                                )
                            # scatter into this batch's a2a input: shard j is
                            # tokens [j*256,(j+1)*256) of batch b; this head
                            # occupies partitions po..po+64 of each shard
                            a2a_view = a2a_in[b][:, :].rearrange(
                                "(j p) t -> p j t", p=128
                            )
                            j0 = tqh * 4  # 1024-wide slab = 4 shards of 256
                            nc.sync.dma_start(
                                out=a2a_view[po:po + DH, j0:j0 + 4, :],
                                in_=fo.rearrange("p (j t) -> p j t", j=4),
                            )

                    # ---- stage C: AllToAll of this batch's head-features ----
                    nc.gpsimd.collective_compute(
                        "AllToAll",
                        mybir.AluOpType.bypass,
                        replica_groups=[list(range(N_CORES))],
                        ins=[a2a_in[b][:, :].opt()],
                        outs=[a2a_out[b][:, :].opt()],
                    )
                    nc.sync.dma_start(
                        out=g_sb[b],
                        in_=a2a_out[b][:, :].rearrange("(j p) t -> p j t", p=128),
                    )

            # ---- stage D: out projection, one pass per batch slice ----
            for ot in range(ET):
                wo_sb = wo_pool.tile([128, ET, 128], BF16)
                nc.sync.dma_start(
                    out=wo_sb,
                    in_=wout_ap[:, ot * 128:(ot + 1) * 128].rearrange(
                        "(ht p) o -> p ht o", p=128
                    ),
                )
                for b in range(2):
                    pp = mmps.tile([128, NW], F32, tag="mm", name="pp_out")
                    for ht in range(ET):
                        nc.tensor.matmul(
                            pp,
                            wo_sb[:, ht, :],
                            g_sb[b][:, ht, :],
                            start=(ht == 0),
                            stop=(ht == ET - 1),
                        )
                    ob = vt_pool.tile([128, NW], F32, tag="ob", name="ob")
                    nc.vector.tensor_scalar_add(ob, pp, bout_sb[:, ot:ot + 1])
                    for tt in range(2):
                        ps = mmps.tile([128, 128], F32, tag="mm", name="ps_otr")
                        nc.tensor.transpose(
                            ps, ob[:, tt * 128:(tt + 1) * 128], ident32
                        )
                        on = small_pool.tile([128, 128], F32, tag="on", name="on")
                        nc.vector.tensor_copy(on, ps)
                        nc.sync.dma_start(
                            out=out_ap[b * NW + tt * 128:b * NW + (tt + 1) * 128,
                                       ot * 128:(ot + 1) * 128],
                            in_=on,
                        )
    nc.compile()
    return nc


def shard_inputs(x, w_qkv, b_qkv, w_out, b_out):
    """Split full inputs into the 8 per-core input maps (bf16 compute)."""
    x2d = np.ascontiguousarray(x.reshape(T, D).astype(np.float32)).astype(BF)
    w_qkv = np.asarray(w_qkv, dtype=np.float32)
    b_qkv = np.asarray(b_qkv, dtype=np.float32)
    w_out = np.ascontiguousarray(np.asarray(w_out, dtype=np.float32).astype(BF))
    b_out = np.asarray(b_out, dtype=np.float32)
    bout_r = np.ascontiguousarray(b_out.reshape(ET, 128).T)  # [p, ot]
    in_maps = []
    for i in range(N_CORES):
        c0 = i * CW
        wq = np.ascontiguousarray(w_qkv[:, c0:c0 + CW] * SCALE).astype(BF)
        wk = np.ascontiguousarray(w_qkv[:, D + c0:D + c0 + CW]).astype(BF)
        wv = np.ascontiguousarray(w_qkv[:, 2 * D + c0:2 * D + c0 + CW]).astype(BF)
        bq = (b_qkv[c0:c0 + CW] * SCALE).reshape(CW, 1)
        bk = b_qkv[D + c0:D + c0 + CW].reshape(CW, 1)
        bv = b_qkv[2 * D + c0:2 * D + c0 + CW].reshape(CW, 1)
        in_maps.append({
            "x": x2d,
            "wq": wq, "wk": wk, "wv": wv,
            "bq": np.ascontiguousarray(bq),
            "bk": np.ascontiguousarray(bk),
            "bv": np.ascontiguousarray(bv),
            "wout": w_out,
            "bout": bout_r,
        })
    return in_maps


def get_nc():
    global _CACHED_NC
    if _CACHED_NC is None:
        _CACHED_NC = build()
    return _CACHED_NC


def run(in_maps, trace=False, **kw):
    nc = get_nc()
    return run_bass_kernel_spmd(
        nc, in_maps, core_ids=list(range(N_CORES)), trace=trace, **kw
    )


def assemble(results):
    """Each core returns [512, 1024]: rows 0..255 = its 256-token slice of
    batch 0, rows 256..511 = its slice of batch 1."""
    out = np.empty((T, D), dtype=np.float32)
    for i, r in enumerate(results):
        o = r["out"]
        out[i * NW:(i + 1) * NW] = o[:NW]
        out[S + i * NW:S + (i + 1) * NW] = o[NW:]
    return out.reshape(B, S, D)


def kernel(x, w_qkv, b_qkv, w_out, b_out):
    in_maps = shard_inputs(x, w_qkv, b_qkv, w_out, b_out)
    res = run(in_maps, trace=False)
    return assemble(res.results)


# revision 23
# speedup vs baseline: 1.6808x; 1.6808x over previous
"""Multi-head attention (embed 1024, 16 heads x 64) on 8 TRN2 NeuronCores.

Sharding: tensor-parallel over heads — each core owns 2 heads end-to-end
(qkv projection columns + attention), then per-batch AllToAlls redistribute
the per-head attention outputs so each core computes the out-projection for
its 256-token slice of each batch.

Compute is bf16 on the TensorEngine (fp32 PSUM accumulation). The engines
execute statically-ordered streams, so the emission order interleaves
independent work (x-transpose/projection chunks, out-projection slices)
between attention iterations to keep the PE dense while ScalarE runs the
softmax exps. Layout choices:
  - x is PE-transposed to xT [e, t] so every projection contracts e on
    partitions; projections produce Q/K/V TRANSPOSED [head_dim, t].
  - scores are computed transposed: St[tk, tq] = Kt.T @ Qt, so the softmax
    sum over tk comes free from a ones-column appended to V: the PV matmul
    outputs [65, tq] with row 64 = sum of exp.
  - normalization: copy numerator+denominator to SBUF (releases the PV PSUM
    accumulator), then reciprocal + K=1 broadcast matmul + vector multiply,
    all off the critical path.
  - out_proj consumes the AllToAll output directly (head-dim on partitions)
    and the result is PE-transposed back to row-major before the output DMA.
"""

import numpy as np
import ml_dtypes

import concourse.bass as bass
import concourse.tile as tile
from concourse import bacc, mybir
from concourse.bass_utils import run_bass_kernel_spmd
from concourse.masks import make_identity

N_CORES = 8
B, S, D = 2, 2048, 1024
T = B * S              # 4096 flattened tokens
HEADS = 16
DH = 64                # head dim
HPC = HEADS // N_CORES  # heads per core = 2
CW = HPC * DH          # per-core qkv width = 128
SCALE = DH ** -0.5
TC = T // N_CORES      # per-core output rows = 512 (256 per batch)
NW = TC // 2           # tokens per batch slice = 256
ET = D // 128          # e partition tiles = 8
F32 = mybir.dt.float32
BF16 = mybir.dt.bfloat16
EXP = mybir.ActivationFunctionType.Exp
BF = ml_dtypes.bfloat16

_CACHED_NC = None


def build():
    nc = bacc.Bacc(
        "TRN2",
        target_bir_lowering=False,
        debug=False,
        num_devices=N_CORES,
    )
    x_ap = nc.dram_tensor("x", [T, D], BF16, kind="ExternalInput").ap()
    wq_ap = nc.dram_tensor("wq", [D, CW], BF16, kind="ExternalInput").ap()
    wk_ap = nc.dram_tensor("wk", [D, CW], BF16, kind="ExternalInput").ap()
    wv_ap = nc.dram_tensor("wv", [D, CW], BF16, kind="ExternalInput").ap()
    bq_ap = nc.dram_tensor("bq", [CW, 1], F32, kind="ExternalInput").ap()
    bk_ap = nc.dram_tensor("bk", [CW, 1], F32, kind="ExternalInput").ap()
    bv_ap = nc.dram_tensor("bv", [CW, 1], F32, kind="ExternalInput").ap()
    wout_ap = nc.dram_tensor("wout", [D, D], BF16, kind="ExternalInput").ap()
    bout_ap = nc.dram_tensor("bout", [128, ET], F32, kind="ExternalInput").ap()
    out_ap = nc.dram_tensor("out", [TC, D], F32, kind="ExternalOutput").ap()

    with tile.TileContext(nc) as tc:
        with (
            tc.tile_pool(name="singles", bufs=1) as singles,
            tc.tile_pool(name="xn", bufs=3) as xn_pool,
            tc.tile_pool(name="xt", bufs=2) as xt_pool,
            tc.tile_pool(name="vt", bufs=2) as vt_pool,
            tc.tile_pool(name="exp", bufs=4) as exp_pool,
            tc.tile_pool(name="fo", bufs=2) as fo_pool,
            tc.tile_pool(name="small", bufs=2) as small_pool,
            tc.tile_pool(name="mmps", bufs=2, space="PSUM") as mmps,
            tc.tile_pool(name="stps", bufs=2, space="PSUM") as stps,
            tc.tile_pool(name="pvps", bufs=1, space="PSUM") as pvps,
            tc.tile_pool(name="dram", bufs=1, space="DRAM") as dram,
        ):
            # ---- A2A bounce buffers, one pair per batch (collectives need
            # internal DRAM). Shard j = tokens [j*256,(j+1)*256) of batch b.
            a2a_in = [dram.tile([D, NW], BF16, name=f"a2a_in{b}") for b in range(2)]
            a2a_out = [dram.tile([D, NW], BF16, name=f"a2a_out{b}") for b in range(2)]

            # ---- constants / weights resident in SBUF ----
            identb = singles.tile([128, 128], BF16)
            make_identity(nc, identb)
            ident32 = singles.tile([128, 128], F32)
            make_identity(nc, ident32)
            ones64 = singles.tile([1, DH], BF16)
            nc.vector.memset(ones64, 1.0)

            w_sb, b_sb = {}, {}
            for name, wap, bap in (
                ("q", wq_ap, bq_ap), ("k", wk_ap, bk_ap), ("v", wv_ap, bv_ap)
            ):
                w_sb[name] = singles.tile(
                    [128, ET, CW], BF16, tag=f"w{name}", name=f"w{name}_sb"
                )
                nc.sync.dma_start(
                    out=w_sb[name],
                    in_=wap.rearrange("(et p) c -> p et c", p=128),
                )
                b_sb[name] = singles.tile(
                    [CW, 1], F32, tag=f"b{name}", name=f"b{name}_sb"
                )
                nc.sync.dma_start(out=b_sb[name], in_=bap)
            bout_sb = singles.tile([128, ET], F32)
            nc.sync.dma_start(out=bout_sb, in_=bout_ap)
            wout_sb = singles.tile([128, ET, D], BF16, tag="wout")
            nc.sync.dma_start(
                out=wout_sb, in_=wout_ap.rearrange("(ht p) o -> p ht o", p=128)
            )

            # persistent activations
            qt = singles.tile([CW, T], BF16, tag="qt")   # [2h*64, t] transposed Q
            kt = singles.tile([CW, T], BF16, tag="kt")
            # V natural per head, 65-wide tk-tiles (col 64 = ones for denom)
            vsb = [
                singles.tile(
                    [128, T // 128, DH + 1], BF16, tag=f"v{h}", name=f"v{h}_sb"
                )
                for h in range(HPC)
            ]
            for h in range(HPC):
                nc.vector.memset(vsb[h][:, :, DH:DH + 1], 1.0)
            # gathered head-features for this core's token rows, per batch
            g_sb = [
                singles.tile([128, ET, NW], BF16, tag=f"g{b}", name=f"g{b}_sb")
                for b in range(2)
            ]

            def emit_chunk(tch):
                """x-transpose + qkv projections for a 512-token chunk."""
                xt_sb = xt_pool.tile([128, ET, 512], BF16, name="xt_sb")
                for tt in range(4):
                    xn = xn_pool.tile([128, D], BF16, name="xn")
                    row0 = tch * 512 + tt * 128
                    nc.sync.dma_start(out=xn, in_=x_ap[row0:row0 + 128, :])
                    for et in range(ET):
                        ps = mmps.tile([128, 128], BF16, tag="mm", name="ps_tr")
                        nc.tensor.transpose(
                            ps, xn[:, et * 128:(et + 1) * 128], identb
                        )
                        nc.vector.tensor_copy(
                            xt_sb[:, et, tt * 128:(tt + 1) * 128], ps
                        )
                for name, dest in (("q", qt), ("k", kt), ("v", None)):
                    pp = mmps.tile([CW, 512], F32, tag="mm", name="pp_proj")
                    for et in range(ET):
                        nc.tensor.matmul(
                            pp,
                            w_sb[name][:, et, :],
                            xt_sb[:, et, :],
                            start=(et == 0),
                            stop=(et == ET - 1),
                        )
                    if dest is not None:
                        nc.vector.tensor_scalar_add(
                            dest[:, tch * 512:(tch + 1) * 512], pp, b_sb[name]
                        )
                    else:
                        vt_tmp = vt_pool.tile([CW, 512], BF16, name="vt_tmp")
                        nc.vector.tensor_scalar_add(vt_tmp, pp, b_sb[name])
                        for tt in range(4):
                            ps2 = mmps.tile([128, 128], BF16, tag="mm", name="ps_vtr")
                            nc.tensor.transpose(
                                ps2, vt_tmp[:, tt * 128:(tt + 1) * 128], identb
                            )
                            ttg = tch * 4 + tt
                            for h in range(HPC):
                                nc.vector.tensor_copy(
                                    vsb[h][:, ttg, 0:DH],
                                    ps2[:, h * DH:(h + 1) * DH],
                                )

            def emit_attention(b, h, tqh):
                """One attention iteration: 1024 queries of head h, batch b."""
                po = h * DH
                tq0 = b * S + tqh * 1024
                pv = pvps.tile([DH + 1, 1024], F32, name="pv")
                for tkt in range(16):
                    st = stps.tile([128, 1024], F32, tag="st", name="st")
                    k0 = b * S + tkt * 128
                    for nh in range(2):
                        nc.tensor.matmul(
                            st[:, nh * 512:(nh + 1) * 512],
                            kt[po:po + DH, k0:k0 + 128],
                            qt[po:po + DH, tq0 + nh * 512:tq0 + (nh + 1) * 512],
                        )
                    ex = exp_pool.tile([128, 1024], BF16, name="ex")
                    nc.scalar.activation(ex, st, EXP)
                    for nh in range(2):
                        nc.tensor.matmul(
                            pv[:, nh * 512:(nh + 1) * 512],
                            vsb[h][:, b * 16 + tkt, :],
                            ex[:, nh * 512:(nh + 1) * 512],
                            start=(tkt == 0),
                            stop=(tkt == 15),
                        )
                # copy numerator + denominator off PSUM so pv frees for the
                # next iteration; normalization runs off the critical path
                pvc = fo_pool.tile([DH, 1024], F32, tag="pvc", name="pvc")
                nc.vector.tensor_copy(pvc, pv[0:DH, :])
                dn = small_pool.tile([1, 1024], F32, tag="dn", name="dn")
                nc.vector.tensor_copy(dn, pv[DH:DH + 1, :])
                recip = small_pool.tile([1, 1024], BF16, name="recip")
                with nc.allow_low_precision(
                    reason="softmax denom reciprocal feeds bf16 bcast"
                ):
                    nc.vector.reciprocal(recip, dn)
                fo = fo_pool.tile([DH, 1024], BF16, name="fo")
                for nh in range(2):
                    bc = mmps.tile([DH, 512], F32, tag="mm", name="bc")
                    nc.tensor.matmul(
                        bc, ones64, recip[:, nh * 512:(nh + 1) * 512]
                    )
                    nc.vector.tensor_mul(
                        fo[:, nh * 512:(nh + 1) * 512],
                        pvc[:, nh * 512:(nh + 1) * 512],
                        bc,
                    )
                # scatter into this batch's a2a input: shard j holds tokens
                # [j*256,(j+1)*256); this head is partitions po..po+64
                a2a_view = a2a_in[b][:, :].rearrange("(j p) t -> p j t", p=128)
                j0 = tqh * 4
                nc.sync.dma_start(
                    out=a2a_view[po:po + DH, j0:j0 + 4, :],
                    in_=fo.rearrange("p (j t) -> p j t", j=4),
                )

            def emit_a2a(b):
                nc.gpsimd.collective_compute(
                    "AllToAll",
                    mybir.AluOpType.bypass,
                    replica_groups=[list(range(N_CORES))],
                    ins=[a2a_in[b][:, :].opt()],
                    outs=[a2a_out[b][:, :].opt()],
                )
                nc.sync.dma_start(
                    out=g_sb[b],
                    in_=a2a_out[b][:, :].rearrange("(j p) t -> p j t", p=128),
                )

            def emit_outproj(b, ot):
                """Out-projection columns [ot*128,(ot+1)*128) for batch b."""
                pp = mmps.tile([128, NW], F32, tag="mm", name="pp_out")
                for ht in range(ET):
                    nc.tensor.matmul(
                        pp,
                        wout_sb[:, ht, ot * 128:(ot + 1) * 128],
                        g_sb[b][:, ht, :],
                        start=(ht == 0),
                        stop=(ht == ET - 1),
                    )
                ob = vt_pool.tile([128, NW], F32, tag="ob", name="ob")
                nc.vector.tensor_scalar_add(ob, pp, bout_sb[:, ot:ot + 1])
                for tt in range(2):
                    ps = mmps.tile([128, 128], F32, tag="mm", name="ps_otr")
                    nc.tensor.transpose(
                        ps, ob[:, tt * 128:(tt + 1) * 128], ident32
                    )
                    on = small_pool.tile([128, 128], F32, tag="on", name="on")
                    nc.vector.tensor_copy(on, ps)
                    nc.sync.dma_start(
                        out=out_ap[b * NW + tt * 128:b * NW + (tt + 1) * 128,
                                   ot * 128:(ot + 1) * 128],
                        in_=on,
                    )

            # ---- emission schedule: weave independent PE work between the
            # serial attention chains so the PE never starves ----
            ATT = [(h, tqh) for h in range(HPC) for tqh in range(2)]
            for tch in range(4):
                emit_chunk(tch)
            for i, (h, tqh) in enumerate(ATT):
                emit_attention(0, h, tqh)
                emit_chunk(4 + i)
            emit_a2a(0)
            for i, (h, tqh) in enumerate(ATT):
                emit_attention(1, h, tqh)
                emit_outproj(0, 2 * i)
                emit_outproj(0, 2 * i + 1)
            emit_a2a(1)
            for ot in range(ET):
                emit_outproj(1, ot)
    nc.compile()
    return nc


def shard_inputs(x, w_qkv, b_qkv, w_out, b_out):
    """Split full inputs into the 8 per-core input maps (bf16 compute)."""
    x2d = np.ascontiguousarray(x.reshape(T, D).astype(np.float32)).astype(BF)
    w_qkv = np.asarray(w_qkv, dtype=np.float32)
    b_qkv = np.asarray(b_qkv, dtype=np.float32)
    w_out = np.ascontiguousarray(np.asarray(w_out, dtype=np.float32).astype(BF))
    b_out = np.asarray(b_out, dtype=np.float32)
    bout_r = np.ascontiguousarray(b_out.reshape(ET, 128).T)  # [p, ot]
    in_maps = []
    for i in range(N_CORES):
        c0 = i * CW
        wq = np.ascontiguousarray(w_qkv[:, c0:c0 + CW] * SCALE).astype(BF)
        wk = np.ascontiguousarray(w_qkv[:, D + c0:D + c0 + CW]).astype(BF)
        wv = np.ascontiguousarray(w_qkv[:, 2 * D + c0:2 * D + c0 + CW]).astype(BF)
        bq = (b_qkv[c0:c0 + CW] * SCALE).reshape(CW, 1)
        bk = b_qkv[D + c0:D + c0 + CW].reshape(CW, 1)
        bv = b_qkv[2 * D + c0:2 * D + c0 + CW].reshape(CW, 1)
        in_maps.append({
            "x": x2d,
            "wq": wq, "wk": wk, "wv": wv,
            "bq": np.ascontiguousarray(bq),
            "bk": np.ascontiguousarray(bk),
            "bv": np.ascontiguousarray(bv),
            "wout": w_out,
            "bout": bout_r,
        })
    return in_maps


def get_nc():
    global _CACHED_NC
    if _CACHED_NC is None:
        _CACHED_NC = build()
    return _CACHED_NC


def run(in_maps, trace=False, **kw):
    nc = get_nc()
    return run_bass_kernel_spmd(
        nc, in_maps, core_ids=list(range(N_CORES)), trace=trace, **kw
    )


def assemble(results):
    """Each core returns [512, 1024]: rows 0..255 = its 256-token slice of
    batch 0, rows 256..511 = its slice of batch 1."""
    out = np.empty((T, D), dtype=np.float32)
    for i, r in enumerate(results):
        o = r["out"]
        out[i * NW:(i + 1) * NW] = o[:NW]
        out[S + i * NW:S + (i + 1) * NW] = o[NW:]
    return out.reshape(B, S, D)


def kernel(x, w_qkv, b_qkv, w_out, b_out):
    in_maps = shard_inputs(x, w_qkv, b_qkv, w_out, b_out)
    res = run(in_maps, trace=False)
    return assemble(res.results)


# revision 24
# speedup vs baseline: 1.9869x; 1.1822x over previous
"""Multi-head attention (embed 1024, 16 heads x 64) on 8 TRN2 NeuronCores.

Sharding: tensor-parallel over heads — each core owns 2 heads end-to-end
(qkv projection columns + attention), then per-(batch, head) AllToAlls
redistribute the per-head attention outputs so each core computes the
out-projection for its 256-token slice of each batch.

Compute is bf16 on the TensorEngine (fp32 PSUM accumulation). The engines
execute statically-ordered instruction streams, so emission order is
software-pipelined: projection chunks and out-projection slices are woven
between attention iterations, and each iteration's softmax normalization
(reciprocal + broadcast + multiply) is deferred by one slot so the slow
one-partition reciprocal never blocks the PE or DVE streams.

Layout:
  - host passes x TRANSPOSED [e, t] so projections contract e on partitions
    with no on-device transpose; projections produce Q/K/V as [head_dim, t].
  - scores are computed transposed: St[tk, tq] = Kt.T @ Qt, so the softmax
    sum over tk comes free from a ones-column appended to V: the PV matmul
    outputs [65, tq] with row 64 = sum of exp.
  - normalization: numerator+denominator copied to SBUF (frees the PV PSUM
    accumulator), then reciprocal + K=1 broadcast matmul + vector multiply.
  - out_proj consumes the AllToAll output directly (head-dim on partitions)
    and the result is PE-transposed back to row-major before the output DMA.
"""

import numpy as np
import ml_dtypes

import concourse.bass as bass
import concourse.tile as tile
from concourse import bacc, mybir
from concourse.bass_utils import run_bass_kernel_spmd
from concourse.masks import make_identity

N_CORES = 8
B, S, D = 2, 2048, 1024
T = B * S              # 4096 flattened tokens
HEADS = 16
DH = 64                # head dim
HPC = HEADS // N_CORES  # heads per core = 2
CW = HPC * DH          # per-core qkv width = 128
SCALE = DH ** -0.5
TC = T // N_CORES      # per-core output rows = 512 (256 per batch)
NW = TC // 2           # tokens per batch slice = 256
ET = D // 128          # e partition tiles = 8
F32 = mybir.dt.float32
BF16 = mybir.dt.bfloat16
EXP = mybir.ActivationFunctionType.Exp
BF = ml_dtypes.bfloat16

_CACHED_NC = None


def build():
    nc = bacc.Bacc(
        "TRN2",
        target_bir_lowering=False,
        debug=False,
        num_devices=N_CORES,
    )
    xt_ap = nc.dram_tensor("xt", [D, T], BF16, kind="ExternalInput").ap()
    wq_ap = nc.dram_tensor("wq", [D, CW], BF16, kind="ExternalInput").ap()
    wk_ap = nc.dram_tensor("wk", [D, CW], BF16, kind="ExternalInput").ap()
    wv_ap = nc.dram_tensor("wv", [D, CW], BF16, kind="ExternalInput").ap()
    bq_ap = nc.dram_tensor("bq", [CW, 1], F32, kind="ExternalInput").ap()
    bk_ap = nc.dram_tensor("bk", [CW, 1], F32, kind="ExternalInput").ap()
    bv_ap = nc.dram_tensor("bv", [CW, 1], F32, kind="ExternalInput").ap()
    wout_ap = nc.dram_tensor("wout", [D, D], BF16, kind="ExternalInput").ap()
    bout_ap = nc.dram_tensor("bout", [128, ET], F32, kind="ExternalInput").ap()
    out_ap = nc.dram_tensor("out", [TC, D], F32, kind="ExternalOutput").ap()

    with tile.TileContext(nc) as tc:
        with (
            tc.tile_pool(name="singles", bufs=1) as singles,
            tc.tile_pool(name="xt", bufs=2) as xt_pool,
            tc.tile_pool(name="vt", bufs=2) as vt_pool,
            tc.tile_pool(name="exp", bufs=4) as exp_pool,
            tc.tile_pool(name="fo", bufs=2) as fo_pool,
            tc.tile_pool(name="small", bufs=2) as small_pool,
            tc.tile_pool(name="mmps", bufs=2, space="PSUM") as mmps,
            tc.tile_pool(name="stps", bufs=2, space="PSUM") as stps,
            tc.tile_pool(name="pvps", bufs=1, space="PSUM") as pvps,
            tc.tile_pool(name="dram", bufs=1, space="DRAM") as dram,
        ):
            # A2A bounce buffers, one pair per (batch, head): shard j holds
            # tokens [j*256,(j+1)*256) of batch b, 64 head-dims per shard.
            a2a_in = [
                [dram.tile([N_CORES * DH, NW], BF16, name=f"a2a_in{b}_{h}")
                 for h in range(HPC)] for b in range(2)
            ]
            a2a_out = [
                [dram.tile([N_CORES * DH, NW], BF16, name=f"a2a_out{b}_{h}")
                 for h in range(HPC)] for b in range(2)
            ]

            # ---- constants / weights resident in SBUF ----
            identb = singles.tile([128, 128], BF16)
            make_identity(nc, identb)
            ident32 = singles.tile([128, 128], F32)
            make_identity(nc, ident32)
            ones64 = singles.tile([1, DH], BF16)
            nc.vector.memset(ones64, 1.0)

            w_sb, b_sb = {}, {}
            for name, wap, bap in (
                ("q", wq_ap, bq_ap), ("k", wk_ap, bk_ap), ("v", wv_ap, bv_ap)
            ):
                w_sb[name] = singles.tile(
                    [128, ET, CW], BF16, tag=f"w{name}", name=f"w{name}_sb"
                )
                nc.gpsimd.dma_start(
                    out=w_sb[name],
                    in_=wap.rearrange("(et p) c -> p et c", p=128),
                )
                b_sb[name] = singles.tile(
                    [CW, 1], F32, tag=f"b{name}", name=f"b{name}_sb"
                )
                nc.gpsimd.dma_start(out=b_sb[name], in_=bap)
            bout_sb = singles.tile([128, ET], F32)
            nc.gpsimd.dma_start(out=bout_sb, in_=bout_ap)
            wout_sb = singles.tile([128, ET, D], BF16, tag="wout")
            nc.gpsimd.dma_start(
                out=wout_sb, in_=wout_ap.rearrange("(ht p) o -> p ht o", p=128)
            )

            # persistent activations
            qt = singles.tile([CW, T], BF16, tag="qt")   # [2h*64, t] transposed Q
            kt = singles.tile([CW, T], BF16, tag="kt")
            # V natural per head, 65-wide tk-tiles (col 64 = ones for denom)
            vsb = [
                singles.tile(
                    [128, T // 128, DH + 1], BF16, tag=f"v{h}", name=f"v{h}_sb"
                )
                for h in range(HPC)
            ]
            for h in range(HPC):
                nc.vector.memset(vsb[h][:, :, DH:DH + 1], 1.0)
            # gathered head-features for this core's token rows, per batch
            g_sb = [
                singles.tile([128, ET, NW], BF16, tag=f"g{b}", name=f"g{b}_sb")
                for b in range(2)
            ]

            xt_view = xt_ap.rearrange("(et p) t -> p et t", p=128)

            def emit_chunk(tch):
                """qkv projections for a 512-token chunk."""
                xt_sb = xt_pool.tile([128, ET, 512], BF16, name="xt_sb")
                nc.sync.dma_start(
                    out=xt_sb,
                    in_=xt_view[:, :, tch * 512:(tch + 1) * 512],
                )
                for name, dest in (("q", qt), ("k", kt), ("v", None)):
                    pp = mmps.tile([CW, 512], F32, tag="mm", name="pp_proj")
                    for et in range(ET):
                        nc.tensor.matmul(
                            pp,
                            w_sb[name][:, et, :],
                            xt_sb[:, et, :],
                            start=(et == 0),
                            stop=(et == ET - 1),
                        )
                    if dest is not None:
                        nc.vector.tensor_scalar_add(
                            dest[:, tch * 512:(tch + 1) * 512], pp, b_sb[name]
                        )
                    else:
                        vt_tmp = vt_pool.tile([CW, 512], BF16, name="vt_tmp")
                        nc.vector.tensor_scalar_add(vt_tmp, pp, b_sb[name])
                        for tt in range(4):
                            ps2 = mmps.tile([128, 128], BF16, tag="mm", name="ps_vtr")
                            nc.tensor.transpose(
                                ps2, vt_tmp[:, tt * 128:(tt + 1) * 128], identb
                            )
                            ttg = tch * 4 + tt
                            for h in range(HPC):
                                nc.vector.tensor_copy(
                                    vsb[h][:, ttg, 0:DH],
                                    ps2[:, h * DH:(h + 1) * DH],
                                )

            def emit_attention(b, h, tqh):
                """One attention iteration: 1024 queries of head h, batch b.
                Returns (pvc, dn) for the deferred normalization."""
                po = h * DH
                tq0 = b * S + tqh * 1024
                pv = pvps.tile([DH + 1, 1024], F32, name="pv")
                for tkt in range(16):
                    st = stps.tile([128, 1024], F32, tag="st", name="st")
                    k0 = b * S + tkt * 128
                    for nh in range(2):
                        nc.tensor.matmul(
                            st[:, nh * 512:(nh + 1) * 512],
                            kt[po:po + DH, k0:k0 + 128],
                            qt[po:po + DH, tq0 + nh * 512:tq0 + (nh + 1) * 512],
                        )
                    ex = exp_pool.tile([128, 1024], BF16, name="ex")
                    nc.scalar.activation(ex, st, EXP)
                    for nh in range(2):
                        nc.tensor.matmul(
                            pv[:, nh * 512:(nh + 1) * 512],
                            vsb[h][:, b * 16 + tkt, :],
                            ex[:, nh * 512:(nh + 1) * 512],
                            start=(tkt == 0),
                            stop=(tkt == 15),
                        )
                # copy numerator + denominator off PSUM so pv frees now
                pvc = fo_pool.tile([DH, 1024], F32, tag="pvc", name="pvc")
                nc.vector.tensor_copy(pvc, pv[0:DH, :])
                dn = small_pool.tile([1, 1024], F32, tag="dn", name="dn")
                nc.vector.tensor_copy(dn, pv[DH:DH + 1, :])
                return pvc, dn

            def emit_finish(b, h, tqh, pvc, dn):
                """Deferred normalization + scatter into the A2A input."""
                po = h * DH
                recip = small_pool.tile([1, 1024], BF16, name="recip")
                with nc.allow_low_precision(
                    reason="softmax denom reciprocal feeds bf16 bcast"
                ):
                    nc.vector.reciprocal(recip, dn)
                fo = fo_pool.tile([DH, 1024], BF16, name="fo")
                for nh in range(2):
                    bc = mmps.tile([DH, 512], F32, tag="mm", name="bc")
                    nc.tensor.matmul(
                        bc, ones64, recip[:, nh * 512:(nh + 1) * 512]
                    )
                    nc.vector.tensor_mul(
                        fo[:, nh * 512:(nh + 1) * 512],
                        pvc[:, nh * 512:(nh + 1) * 512],
                        bc,
                    )
                # head h of batch b goes to a2a_in[b][h]: shard j = rank j's
                # 256 tokens, 64 rows each
                a2a_view = a2a_in[b][h][:, :].rearrange("(j p) t -> p j t", p=DH)
                j0 = tqh * 4
                nc.sync.dma_start(
                    out=a2a_view[:, j0:j0 + 4, :],
                    in_=fo.rearrange("p (j t) -> p j t", j=4),
                )

            def emit_a2a(b, h):
                nc.gpsimd.collective_compute(
                    "AllToAll",
                    mybir.AluOpType.bypass,
                    replica_groups=[list(range(N_CORES))],
                    ins=[a2a_in[b][h][:, :].opt()],
                    outs=[a2a_out[b][h][:, :].opt()],
                )
                # rank r's shard lands at rows r*64..(r+1)*64 → head h's dims
                # are partitions h*64..h*64+64 of g_sb[b][:, r, :]
                nc.sync.dma_start(
                    out=g_sb[b][h * DH:(h + 1) * DH, :, :],
                    in_=a2a_out[b][h][:, :].rearrange("(r p) t -> p r t", p=DH),
                )

            def emit_outproj(b, ot):
                """Out-projection columns [ot*128,(ot+1)*128) for batch b."""
                pp = mmps.tile([128, NW], F32, tag="mm", name="pp_out")
                for ht in range(ET):
                    nc.tensor.matmul(
                        pp,
                        wout_sb[:, ht, ot * 128:(ot + 1) * 128],
                        g_sb[b][:, ht, :],
                        start=(ht == 0),
                        stop=(ht == ET - 1),
                    )
                ob = vt_pool.tile([128, NW], F32, tag="ob", name="ob")
                nc.vector.tensor_scalar_add(ob, pp, bout_sb[:, ot:ot + 1])
                for tt in range(2):
                    ps = mmps.tile([128, 128], F32, tag="mm", name="ps_otr")
                    nc.tensor.transpose(
                        ps, ob[:, tt * 128:(tt + 1) * 128], ident32
                    )
                    on = small_pool.tile([128, 128], F32, tag="on", name="on")
                    nc.vector.tensor_copy(on, ps)
                    nc.sync.dma_start(
                        out=out_ap[b * NW + tt * 128:b * NW + (tt + 1) * 128,
                                   ot * 128:(ot + 1) * 128],
                        in_=on,
                    )

            # ---- emission schedule: software-pipelined. fin(i) lands one
            # attention iteration after att(i) so the reciprocal is ready
            # before its broadcast matmul enters the PE stream. ----
            for tch in range(4):
                emit_chunk(tch)
            ATT0 = [(0, h, tqh) for h in range(HPC) for tqh in range(2)]
            ATT1 = [(1, h, tqh) for h in range(HPC) for tqh in range(2)]
            pend = {}

            # batch 0 attention, interleaved with chunks 4..7
            pend[0] = emit_attention(*ATT0[0])
            emit_chunk(4)
            pend[1] = emit_attention(*ATT0[1])
            emit_finish(*ATT0[0], *pend[0])
            emit_chunk(5)
            pend[2] = emit_attention(*ATT0[2])
            emit_finish(*ATT0[1], *pend[1])
            emit_a2a(0, 0)
            emit_chunk(6)
            pend[3] = emit_attention(*ATT0[3])
            emit_finish(*ATT0[2], *pend[2])
            emit_chunk(7)
            # batch 1 attention, interleaved with batch-0 out-projection
            pend[4] = emit_attention(*ATT1[0])
            emit_finish(*ATT0[3], *pend[3])
            emit_a2a(0, 1)
            pend[5] = emit_attention(*ATT1[1])
            emit_finish(*ATT1[0], *pend[4])
            emit_outproj(0, 0)
            emit_outproj(0, 1)
            pend[6] = emit_attention(*ATT1[2])
            emit_finish(*ATT1[1], *pend[5])
            emit_a2a(1, 0)
            emit_outproj(0, 2)
            emit_outproj(0, 3)
            pend[7] = emit_attention(*ATT1[3])
            emit_finish(*ATT1[2], *pend[6])
            for ot in range(4, 8):
                emit_outproj(0, ot)
            emit_finish(*ATT1[3], *pend[7])
            emit_a2a(1, 1)
            for ot in range(ET):
                emit_outproj(1, ot)
    nc.compile()
    return nc


def shard_inputs(x, w_qkv, b_qkv, w_out, b_out):
    """Split full inputs into the 8 per-core input maps (bf16 compute).
    x is transposed host-side so projections need no on-device transpose."""
    x2d = np.asarray(x, dtype=np.float32).reshape(T, D)
    xt = np.ascontiguousarray(x2d.T.astype(BF))  # [D, T]
    w_qkv = np.asarray(w_qkv, dtype=np.float32)
    b_qkv = np.asarray(b_qkv, dtype=np.float32)
    w_out = np.ascontiguousarray(np.asarray(w_out, dtype=np.float32).astype(BF))
    b_out = np.asarray(b_out, dtype=np.float32)
    bout_r = np.ascontiguousarray(b_out.reshape(ET, 128).T)  # [p, ot]
    in_maps = []
    for i in range(N_CORES):
        c0 = i * CW
        wq = np.ascontiguousarray(w_qkv[:, c0:c0 + CW] * SCALE).astype(BF)
        wk = np.ascontiguousarray(w_qkv[:, D + c0:D + c0 + CW]).astype(BF)
        wv = np.ascontiguousarray(w_qkv[:, 2 * D + c0:2 * D + c0 + CW]).astype(BF)
        bq = (b_qkv[c0:c0 + CW] * SCALE).reshape(CW, 1)
        bk = b_qkv[D + c0:D + c0 + CW].reshape(CW, 1)
        bv = b_qkv[2 * D + c0:2 * D + c0 + CW].reshape(CW, 1)
        in_maps.append({
            "xt": xt,
            "wq": wq, "wk": wk, "wv": wv,
            "bq": np.ascontiguousarray(bq),
            "bk": np.ascontiguousarray(bk),
            "bv": np.ascontiguousarray(bv),
            "wout": w_out,
            "bout": bout_r,
        })
    return in_maps


def get_nc():
    global _CACHED_NC
    if _CACHED_NC is None:
        _CACHED_NC = build()
    return _CACHED_NC


def run(in_maps, trace=False, **kw):
    nc = get_nc()
    return run_bass_kernel_spmd(
        nc, in_maps, core_ids=list(range(N_CORES)), trace=trace, **kw
    )


def assemble(results):
    """Each core returns [512, 1024]: rows 0..255 = its 256-token slice of
    batch 0, rows 256..511 = its slice of batch 1."""
    out = np.empty((T, D), dtype=np.float32)
    for i, r in enumerate(results):
        o = r["out"]
        out[i * NW:(i + 1) * NW] = o[:NW]
        out[S + i * NW:S + (i + 1) * NW] = o[NW:]
    return out.reshape(B, S, D)


def kernel(x, w_qkv, b_qkv, w_out, b_out):
    in_maps = shard_inputs(x, w_qkv, b_qkv, w_out, b_out)
    res = run(in_maps, trace=False)
    return assemble(res.results)
